# revision 42
# baseline (speedup 1.0000x reference)
"""Trainium2 Bass kernel for nn_Attention_56822417326562 (dense transformer block).

Sharding: data-parallel over batch — core i computes batch element i entirely
(B=8 over 8 NeuronCores, no collectives).

Per-core math (x: [512, 1600]):
  BN folded into weights on host; softmax scale folded into q. All inputs are
  DMAed as channel-grouped [128, 4*m] tensors (one big-packet DMA each) on the
  sync queue, critical tensors first.
  Stage A (PE): q, k, v via 1x1 convs (bf16), then vT built by PE transposes
  of v (identity matmul) with a ones column appended per head for the softmax
  denominator. k is consumed via 32-row tile_position matmuls (no zero
  padding of the contraction dim), biases folded into the DVE PSUM drains.
  Attention (per half of n, per head): scores S^T[m,n] on PE (bf16), exp on
  ScalarE (its only work — the pacing engine), out_un[d,n] and s[n] in one PE
  accumulation via the vT ones column. The mm accumulator is drained by DVE
  copies (zs per head, s row into s_g); 1/s via full-tile DVE reciprocal at
  assembly time (single-partition reciprocal miscomputes on HW).
  pe = depthwise 3x3 as 9 diagonal bf16 matmuls over zero-padded v.
  Assembly (per half): 1/s broadcast via bf16 ones-matmul (tile_position row
  = head), z16 = out_un * (1/s) + pe (DVE), proj on PE, proj bias folded into
  the PSUM->SBUF drain, y DMAed out as bf16 per 400-col block.

HAM clock-gate management (the dominant perf effect): the PE clock sits at
1.2 GHz unless each free-running 3.4us activity window is ~fully busy; any
sparse window re-throttles to K=4/8 and halves PE speed for >=13.6us. The
per-head slot schedule interleaves contiguous ~1.7us PE bursts (the real
depthwise-conv jobs, the previous half's normalize/proj assembly jobs, and a
few discarded dummy bursts) between attention heads so the PE queue always
holds a multi-us backlog. Cross-engine dependencies get at least a one-head
lead so the in-order PE queue never stalls on DVE results. Separately, the
chip has a persistent P-state that can inflate all engines ~18% run to run;
only same-process paired A/B comparisons are meaningful.
"""
import sys

sys.path.insert(0, "/opt/trn_rl_repo")

import numpy as np

DIM = 512
NH = 8
HD = 64
KD = 32
NPOS = 1600
EPS = 1e-5
SCALE = float(KD) ** -0.5
NMT = 13  # position tiles: 12*128 + 64
HALF = 800

_compiled_nc = None


def build_nc(dump=False, f32c1=False, extra_d=1, extra_nmm=9):
    import concourse.tile as tile
    from concourse import bacc, mybir

    f32 = mybir.dt.float32
    f32r = mybir.dt.float32r
    f16 = mybir.dt.float16
    bf16 = mybir.dt.bfloat16
    AF = mybir.ActivationFunctionType
    OP = mybir.AluOpType

    nc = bacc.Bacc("TRN2", target_bir_lowering=False, debug=False, num_devices=8)

    x16_d = nc.dram_tensor("x16", [128, 4 * NPOS], bf16, kind="ExternalInput").ap()
    wq16_d = nc.dram_tensor("wq16", [128, 4 * 256], bf16, kind="ExternalInput").ap()
    wk16_d = nc.dram_tensor("wk16", [128, 4 * 256], bf16, kind="ExternalInput").ap()
    wv16_d = nc.dram_tensor("wv16", [128, 4 * DIM], bf16, kind="ExternalInput").ap()
    wp16_d = nc.dram_tensor("wp16", [128, 4 * DIM], bf16, kind="ExternalInput").ap()
    bq_d = nc.dram_tensor("bq", [128, 2], f32, kind="ExternalInput").ap()
    bk_d = nc.dram_tensor("bk", [128, 2], f32, kind="ExternalInput").ap()
    bv_d = nc.dram_tensor("bv", [128, 4], f32, kind="ExternalInput").ap()
    bp_d = nc.dram_tensor("bp", [128, 4], f32, kind="ExternalInput").ap()
    wpe_d = nc.dram_tensor("wpe", [128, 36], f32, kind="ExternalInput").ap()
    bpe_d = nc.dram_tensor("bpe", [128, 4], f32, kind="ExternalInput").ap()
    ident_d = nc.dram_tensor("ident", [128, 128], bf16, kind="ExternalInput").ap()
    ones_d = nc.dram_tensor("ones", [128, 64], bf16, kind="ExternalInput").ap()
    pdg_d = nc.dram_tensor("pdg", [128, 36 * 128], bf16, kind="ExternalInput").ap()
    y_d = nc.dram_tensor("y", [DIM, NPOS], bf16, kind="ExternalOutput").ap()

    def mt_sz(j):
        return 64 if j == NMT - 1 else 128

    with tile.TileContext(nc) as tc:
        with (
            tc.tile_pool(name="pers", bufs=1) as pers,
            tc.tile_pool(name="ps2", bufs=2, space="PSUM") as ps2,
            tc.tile_pool(name="scp", bufs=2, space="PSUM") as scp,
            tc.tile_pool(name="mmp", bufs=1, space="PSUM") as mmp,
            tc.tile_pool(name="ep", bufs=4) as ep,
            tc.tile_pool(name="ystg", bufs=4) as ystg,
        ):
            x16_all = pers.tile([128, 4 * NPOS], bf16, name="x16_all")
            wq_all = pers.tile([128, 4 * 256], bf16, name="wq_all")
            wk_all = pers.tile([128, 4 * 256], bf16, name="wk_all")
            wv_all = pers.tile([128, 4 * DIM], bf16, name="wv_all")
            wp_all = pers.tile([128, 4 * DIM], bf16, name="wp_all")
            x16_sb = [x16_all[:, NPOS * c : NPOS * (c + 1)] for c in range(4)]
            wq_sb = [wq_all[:, 256 * c : 256 * (c + 1)] for c in range(4)]
            wk_sb = [wk_all[:, 256 * c : 256 * (c + 1)] for c in range(4)]
            wv_sb = [wv_all[:, DIM * c : DIM * (c + 1)] for c in range(4)]
            wp_sb = [wp_all[:, DIM * c : DIM * (c + 1)] for c in range(4)]
            bq_sb = pers.tile([128, 2], f32, name="bq_sb")
            bk_sb = pers.tile([128, 2], f32, name="bk_sb")
            bv_sb = pers.tile([128, 4], f32, name="bv_sb")
            bp_sb = pers.tile([128, 4], f32, name="bp_sb")
            wpe_sb = pers.tile([128, 36], f32, name="wpe_sb")
            bpe_sb = pers.tile([128, 4], f32, name="bpe_sb")
            ident_sb = pers.tile([128, 128], bf16, name="ident_sb")
            ones32 = pers.tile([128, 64], bf16, name="ones32")
            q_hi = [pers.tile([128, NPOS], bf16, name=f"qhi{t}") for t in range(2)]
            k_sb = [pers.tile([128, NPOS], bf16, name=f"k{t}") for t in range(2)]
            if f32c1:
                q32 = [pers.tile([128, NPOS], f32, name=f"q32{t}") for t in range(2)]
                k32 = [pers.tile([128, NPOS], f32, name=f"k32{t}") for t in range(2)]
            v_sb = [pers.tile([128, NPOS], bf16, name=f"v{o}") for o in range(4)]
            vT_sb = [pers.tile([128, NH * 65], bf16, name=f"vT{j}") for j in range(NMT)]
            pe_sb = [pers.tile([128, NPOS], bf16, name=f"pe{t}") for t in range(4)]
            vpad = [pers.tile([128, 42 * 42], bf16, name=f"vpad{t}") for t in range(4)]
            pdg_all = pers.tile([128, 36 * 128], bf16, name="pdg_all")
            pdg_sb = [pdg_all[:, 128 * i : 128 * (i + 1)] for i in range(36)]
            z16 = [pers.tile([128, NPOS], bf16, name=f"z16{t}") for t in range(4)]
            zs = [pers.tile([64, HALF], bf16, name=f"zs{g}") for g in range(16)]
            s_g = [pers.tile([128, NPOS], f32, name=f"s_g{i}") for i in range(2)]
            rbf = [pers.tile([128, NPOS], bf16, name=f"rbf{i}") for i in range(2)]

            # ---- input DMAs: consolidated channel-grouped tensors,
            # x split across two hw queues; critical tensors first ----
            nc.sync.dma_start(x16_all[:], x16_d[:])
            nc.sync.dma_start(wq_all[:], wq16_d[:])
            nc.sync.dma_start(wk_all[:], wk16_d[:])
            nc.sync.dma_start(wv_all[:], wv16_d[:])
            nc.sync.dma_start(ident_sb[:], ident_d[:])
            nc.scalar.dma_start(bq_sb[:], bq_d[:])
            nc.scalar.dma_start(bk_sb[:], bk_d[:])
            nc.scalar.dma_start(bv_sb[:], bv_d[:])
            nc.sync.dma_start(ones32[:], ones_d[:])
            nc.sync.dma_start(wpe_sb[:], wpe_d[:])
            nc.sync.dma_start(bpe_sb[:], bpe_d[:])
            nc.sync.dma_start(wp_all[:], wp16_d[:])
            nc.sync.dma_start(bp_sb[:], bp_d[:])
            nc.sync.dma_start(pdg_all[:], pdg_d[:])

            for i in range(2):
                nc.gpsimd.memset(s_g[i][:], 1.0)
            for t in range(4):
                vg = vpad[t].rearrange("p (a b) -> p a b", a=42)
                nc.gpsimd.memset(vg[:, 0:1, :], 0.0)
                nc.gpsimd.memset(vg[:, 41:42, :], 0.0)
                nc.gpsimd.memset(vg[:, 1:41, 0:1], 0.0)
                nc.gpsimd.memset(vg[:, 1:41, 41:42], 0.0)
            vT_g = [vT_sb[j].rearrange("p (h g) -> p h g", g=65) for j in range(NMT)]
            for j in range(NMT):
                nc.gpsimd.memset(vT_g[j][0 : mt_sz(j), :, 64:65], 1.0)

            # ---- stage A: q, k (bf16, bias via DVE drain) ----
            for w_sb, b_sb, dst in ((wq_sb, bq_sb, q_hi), (wk_sb, bk_sb, k_sb)):
                for t in range(2):
                    for ch in range(4):
                        cs = slice(400 * ch, 400 * (ch + 1))
                        ps = ps2.tile([128, 512], f32, name="psqk", tag="ps2")
                        for c in range(4):
                            nc.tensor.matmul(
                                ps[:, 0:400],
                                w_sb[c][:, 128 * t : 128 * (t + 1)],
                                x16_sb[c][:, cs],
                                start=(c == 0),
                                stop=(c == 3),
                            )
                        nc.vector.tensor_scalar_add(
                            dst[t][:, cs], ps[:, 0:400], b_sb[:, t : t + 1]
                        )
                        if f32c1:
                            dst32 = q32 if dst is q_hi else k32
                            nc.vector.tensor_scalar_add(
                                dst32[t][:, cs], ps[:, 0:400], b_sb[:, t : t + 1]
                            )

            # ---- stage A: v natural, then vT via PE transpose ----
            for o in range(4):
                for ch in range(4):
                    cs = slice(400 * ch, 400 * (ch + 1))
                    ps = ps2.tile([128, 512], f32, name="psv", tag="ps2")
                    for c in range(4):
                        nc.tensor.matmul(
                            ps[:, 0:400],
                            wv_sb[c][:, 128 * o : 128 * (o + 1)],
                            x16_sb[c][:, cs],
                            start=(c == 0),
                            stop=(c == 3),
                        )
                    nc.vector.tensor_scalar_add(
                        v_sb[o][:, cs], ps[:, 0:400], bv_sb[:, o : o + 1]
                    )
                    nc.gpsimd.tensor_copy(
                        vpad[o].rearrange("p (a b) -> p a b", a=42)[
                            :, 1 + 10 * ch : 11 + 10 * ch, 1:41
                        ],
                        v_sb[o][:, cs].rearrange("p (a b) -> p a b", a=10),
                    )

            for j in range(NMT):
                mj = mt_sz(j)
                psT = ps2.tile([128, 512], bf16, name="psT", tag="ps2")
                for t in range(4):
                    nc.tensor.transpose(
                        psT[0:mj, 128 * t : 128 * (t + 1)],
                        v_sb[t][:, 128 * j : 128 * j + mj],
                        ident_sb[:],
                    )
                nc.vector.tensor_copy(
                    vT_g[j][0:mj, :, 0:64],
                    psT[0:mj, :].rearrange("p (h d) -> p h d", d=64),
                )

            # ---- pe: depthwise 3x3 as PE diagonal matmuls. Each (t, ch) job
            # is a contiguous ~1.7us PE burst, scheduled as HAM-warmth filler
            # between attention heads ----
            def make_pe_job(t, ch):
                def pejob():
                    vg = vpad[t].rearrange("p (a b) -> p a b", a=42)
                    ps = ps2.tile([128, 512], f32, name="pspe", tag="ps2")
                    for k9 in range(9):
                        dy, dx = k9 // 3 - 1, k9 % 3 - 1
                        rhs = vg[
                            :, 1 + 10 * ch + dy : 11 + 10 * ch + dy, 1 + dx : 41 + dx
                        ]
                        nc.tensor.matmul(
                            ps[:, 0:400],
                            pdg_sb[9 * t + k9][:],
                            rhs,
                            start=(k9 == 0),
                            stop=(k9 == 8),
                        )
                    nc.vector.tensor_scalar_add(
                        pe_sb[t][:, 400 * ch : 400 * (ch + 1)],
                        ps[:, 0:400],
                        bpe_sb[:, t : t + 1],
                    )

                return pejob

            pe_jobs = {t: [make_pe_job(t, ch) for ch in range(4)] for t in range(4)}

            pe_scr = pers.tile([128, 400], f16, name="pe_scr")

            def make_dummy_job(nmm=9):
                def djob():
                    vg = vpad[0].rearrange("p (a b) -> p a b", a=42)
                    ps = ps2.tile([128, 512], f32, name="psdm", tag="ps2")
                    for k9 in range(nmm):
                        dy, dx = k9 % 3 - 1, k9 // 3 - 1
                        rhs = vg[:, 1 + dy : 11 + dy, 1 + dx : 41 + dx]
                        nc.tensor.matmul(
                            ps[:, 0:400],
                            pdg_sb[k9][:],
                            rhs,
                            start=(k9 == 0),
                            stop=(k9 == nmm - 1),
                        )
                    nc.vector.tensor_copy(pe_scr[:], ps[:, 0:400])

                return djob

            # ---- assembly maker (per half): normalize, +pe, proj, out ----
            def make_assembly(half):
                hs = slice(HALF * half, HALF * (half + 1))
                jobs = []

                def make_sjob(i):
                    def sjob():
                        nc.vector.reciprocal_approx_fast(
                            s_g[i][:, hs], s_g[i][:, hs]
                        )
                        nc.vector.tensor_copy(rbf[i][:, hs], s_g[i][:, hs])

                    return sjob

                def make_tjob(t):
                    def tjob():
                        for i in range(2):
                            h2 = 2 * t + i
                            g = 8 * half + h2
                            sr2 = 32 * (h2 % 4)
                            for off, ncols in ((0, 512), (512, 288)):
                                rb = ps2.tile([128, 512], f32, name="rb", tag="ps2")
                                nc.tensor.matmul(
                                    rb[0:64, 0:ncols],
                                    ones32[sr2 : sr2 + 1, 0:64],
                                    rbf[h2 // 4][
                                        sr2 : sr2 + 1,
                                        HALF * half + off : HALF * half + off + ncols,
                                    ],
                                    tile_position=(sr2, 0),
                                )
                                nc.vector.tensor_tensor(
                                    z16[t][
                                        64 * i : 64 * (i + 1),
                                        HALF * half + off : HALF * half + off + ncols,
                                    ],
                                    zs[g][0:64, off : off + ncols],
                                    rb[0:64, 0:ncols],
                                    op=OP.mult,
                                )
                        nc.vector.tensor_tensor(
                            z16[t][:, hs], z16[t][:, hs], pe_sb[t][:, hs], op=OP.add
                        )

                    return tjob

                def make_pjob(o, ch, drain=None):
                    def pjob():
                        cs = slice(
                            HALF * half + 400 * ch, HALF * half + 400 * (ch + 1)
                        )
                        pj = ps2.tile([128, 512], f32, name="pj", tag="ps2")
                        for c in range(4):
                            nc.tensor.matmul(
                                pj[:, 0:400],
                                wp_sb[c][:, 128 * o : 128 * (o + 1)],
                                z16[c][:, cs],
                                start=(c == 0),
                                stop=(c == 3),
                            )
                        yt = ystg.tile([128, 400], bf16, name="yt", tag="yt")
                        if drain == "scalar":
                            nc.scalar.activation(
                                yt[:], pj[:, 0:400], AF.Identity,
                                bias=bp_sb[:, o : o + 1],
                            )
                        else:
                            nc.vector.tensor_scalar_add(
                                yt[:], pj[:, 0:400], bp_sb[:, o : o + 1]
                            )
                        nc.sync.dma_start(y_d[128 * o : 128 * (o + 1), cs], yt[:])

                    return pjob

                jb = {}
                jb["s0"] = make_sjob(0)
                jb["s1"] = make_sjob(1)
                for t in range(4):
                    jb[f"t{t}"] = make_tjob(t)
                for o in range(4):
                    for ch in range(2):
                        jb[f"p{o}{ch}"] = make_pjob(o, ch)
                        jb[f"P{o}{ch}"] = make_pjob(o, ch, drain="scalar")
                return jb

            # ---- explicit per-head filler/assembly schedule ----
            # Keeps the PE densely busy every head (HAM K=8/8) while honoring
            # cross-engine dependencies with at least a one-head lead time.
            asm0 = make_assembly(0)
            asm1 = make_assembly(1)
            D = make_dummy_job
            slot = {}
            for h in range(4):
                slot[(0, h)] = [pe_jobs[0][h], pe_jobs[1][h]]
            slot[(0, 4)] = [asm0["s0"], pe_jobs[2][0], pe_jobs[2][1]]
            slot[(0, 5)] = [asm0["t0"], pe_jobs[2][2], pe_jobs[2][3]]
            slot[(0, 6)] = [asm0["t1"], pe_jobs[3][0], pe_jobs[3][1]]
            slot[(0, 7)] = [pe_jobs[3][2], pe_jobs[3][3], D()]
            slot[(1, 0)] = [asm0["s1"], D(), D()]
            slot[(1, 1)] = [asm0["t2"], asm0["t3"], D()]
            slot[(1, 2)] = [asm0["p00"], asm0["p01"], D()]
            slot[(1, 3)] = [asm0["p10"], asm0["p11"], D()]
            slot[(1, 4)] = [asm0["p20"], asm0["p21"], asm1["s0"], D()]
            slot[(1, 5)] = [asm0["p30"], asm0["p31"], asm1["t0"], D()]
            slot[(1, 6)] = [asm1["t1"], D(), D()]
            slot[(1, 7)] = [D()]
            for h in range(8):
                slot[(1, h)] += [D(extra_nmm) for _ in range(extra_d)]
            tail = [asm1["s1"], D(), asm1["t2"], asm1["t3"], D(5)] + [
                asm1[f"P{o}{ch}"] for o in range(4) for ch in range(2)
            ]

            for half in range(2):
                hs2 = slice(HALF * half, HALF * (half + 1))
                c0 = slice(HALF * half, HALF * half + 512)
                c1 = slice(HALF * half + 512, HALF * half + 800)
                for h in range(8):
                    t = h // 4
                    sr = 32 * (h % 4)
                    g = 8 * half + h
                    mm = mmp.tile([65, HALF], f32, name="mm", tag="mm")

                    def mm3(j, E):
                        mj = mt_sz(j)
                        lhsT = vT_g[j][0:mj, h, :]
                        nc.tensor.matmul(
                            mm[:, 0:512],
                            lhsT,
                            E[0:mj, 0:512],
                            start=(j == 0),
                            stop=(j == NMT - 1),
                        )
                        nc.tensor.matmul(
                            mm[:, 512:800],
                            lhsT,
                            E[0:mj, 512:800],
                            start=(j == 0),
                            stop=(j == NMT - 1),
                        )

                    prev = None
                    for j in range(NMT):
                        mj = mt_sz(j)
                        ms = slice(128 * j, 128 * j + mj)
                        sc = scp.tile([128, HALF], f32, name="sc", tag="sc")
                        nc.tensor.matmul(
                            sc[0:mj, 0:512],
                            k_sb[t][sr : sr + 32, ms],
                            q_hi[t][sr : sr + 32, c0],
                            tile_position=(sr, 0),
                        )
                        if f32c1:
                            nc.tensor.matmul(
                                sc[0:mj, 512:800],
                                k32[t][sr : sr + 32, ms],
                                q32[t][sr : sr + 32, c1],
                                tile_position=(sr, 0),
                            )
                        else:
                            nc.tensor.matmul(
                                sc[0:mj, 512:800],
                                k_sb[t][sr : sr + 32, ms],
                                q_hi[t][sr : sr + 32, c1],
                                tile_position=(sr, 0),
                            )
                        E = ep.tile([128, HALF], bf16, name="E", tag="E")
                        nc.scalar.activation(E[0:mj, :], sc[0:mj, :], AF.Exp)
                        if prev is not None:
                            mm3(*prev)
                        prev = (j, E)
                    mm3(*prev)
                    nc.vector.tensor_copy(zs[g][:], mm[0:64, :])
                    nc.vector.tensor_copy(s_g[h // 4][sr : sr + 1, hs2], mm[64:65, :])
                    for jobf in slot[(half, h)]:
                        jobf()
            for jobf in tail:
                jobf()

            if dump:
                dbg_specs = [
                    ("q0", q_hi[0]),
                    ("k0", k_sb[0]),
                    ("v0", v_sb[0]),
                    ("vt0", vT_sb[0]),
                    ("pe0", pe_sb[0]),
                    ("zs0", zs[0]),
                    ("z160", z16[0]),
                ]
                for nm, t_sb in dbg_specs:
                    t_d = nc.dram_tensor(
                        f"dbg_{nm}", list(t_sb.shape), t_sb.dtype, kind="ExternalOutput"
                    ).ap()
                    nc.sync.dma_start(t_d[:], t_sb[:])

    nc.compile()
    return nc


def prep_weights(inputs):
    import ml_dtypes

    bfl = ml_dtypes.bfloat16
    d = lambda k: np.asarray(inputs[k], dtype=np.float64)
    inv = d("qkv_gamma") / np.sqrt(d("qkv_var") + EPS)
    W = d("qkv_w") * inv[:, None]
    bb = d("qkv_beta") - d("qkv_mean") * inv
    Wh = W.reshape(NH, 2 * KD + HD, DIM)
    bh = bb.reshape(NH, 2 * KD + HD)
    Wq = (Wh[:, :KD] * SCALE).reshape(NH * KD, DIM)
    bq = (bh[:, :KD] * SCALE).reshape(-1)
    Wk = Wh[:, KD : 2 * KD].reshape(NH * KD, DIM)
    bk = bh[:, KD : 2 * KD].reshape(-1)
    Wv = Wh[:, 2 * KD :].reshape(NH * HD, DIM)
    bv = bh[:, 2 * KD :].reshape(-1)

    ipe = d("pe_gamma") / np.sqrt(d("pe_var") + EPS)
    wpe = d("pe_w")[:, 0] * ipe[:, None, None]  # [512, 3, 3]
    bpe = d("pe_beta") - d("pe_mean") * ipe
    wpe_tap = np.zeros((128, 36), np.float64)
    for t in range(4):
        for k9 in range(9):
            wpe_tap[:, 9 * t + k9] = wpe[128 * t : 128 * (t + 1), k9 // 3, k9 % 3]
    pdg = np.zeros((36, 128, 128), np.float64)
    ar = np.arange(128)
    for t in range(4):
        for k9 in range(9):
            pdg[t * 9 + k9, ar, ar] = wpe[128 * t : 128 * (t + 1), k9 // 3, k9 % 3]

    ip = d("proj_gamma") / np.sqrt(d("proj_var") + EPS)
    Wp = d("proj_w") * ip[:, None]
    bp = d("proj_beta") - d("proj_mean") * ip

    c32 = lambda a: np.ascontiguousarray(a, dtype=np.float32)
    c16 = lambda a: np.ascontiguousarray(a.astype(np.float32), dtype=bfl)

    def grp(wT):
        # [512, m] -> [128, 4*m]: row p = concat over c of wT[128c+p, :]
        m = wT.shape[1]
        return wT.reshape(4, 128, m).transpose(1, 0, 2).reshape(128, 4 * m)

    return dict(
        wq16=c16(grp(Wq.T)),
        wk16=c16(grp(Wk.T)),
        wv16=c16(grp(Wv.T)),
        wp16=c16(grp(Wp.T)),
        bq=c32(bq.reshape(2, 128).T),
        bk=c32(bk.reshape(2, 128).T),
        bv=c32(bv.reshape(4, 128).T),
        bp=c32(bp.reshape(4, 128).T),
        wpe=c32(wpe_tap),
        bpe=c32(bpe.reshape(4, 128).T),
        ident=c16(np.eye(128)),
        ones=c16(np.ones((128, 64))),
        pdg=c16(pdg.transpose(1, 0, 2).reshape(128, 36 * 128)),
    )


def make_in_maps(inputs):
    import ml_dtypes

    w = prep_weights(inputs)
    x = np.asarray(inputs["x"], dtype=np.float32)
    B = x.shape[0]
    maps = []
    for i in range(B):
        xi = x[i].reshape(4, 128, NPOS).transpose(1, 0, 2).reshape(128, 4 * NPOS)
        maps.append({"x16": np.ascontiguousarray(xi).astype(ml_dtypes.bfloat16), **w})
    return maps


def kernel(**inputs):
    global _compiled_nc
    from concourse.bass_utils import run_bass_kernel_spmd

    if _compiled_nc is None:
        _compiled_nc = build_nc()
    in_maps = make_in_maps(inputs)
    res = run_bass_kernel_spmd(_compiled_nc, in_maps, core_ids=list(range(8)))
    y = np.stack(
        [
            np.asarray(res.results[i]["y"], dtype=np.float32).reshape(DIM, 40, 40)
            for i in range(8)
        ]
    )
    return y


if __name__ == "__main__":
    nc = build_nc()
    print("built ok")
